# revision 1
# baseline (speedup 1.0000x reference)
"""GNN message-passing kernel (max+mean aggregation -> linear -> log_softmax)
for Trainium2, 8 NeuronCores, dst-node sharding.

Strategy:
- Shard destination nodes: core c owns global nodes [c*12500, (c+1)*12500),
  padded to 12544 = 98*128 local slots.
- Host sorts each core's nodes by in-degree and builds a SHARED degree
  template T[p] = max over cores of the p-th sorted degree, so one SPMD
  program serves all 8 cores; per-core index data pads missing slots with a
  neutral row.
- Neighbor features are gathered on-device with indirect DMA (int32 row
  indices) from xg = concat([zeros row], x + SHIFT). The shift makes the
  zero pad row neutral for max; pads add exactly 0 to sums; the shift is
  cancelled exactly by folding -SHIFT * rowsum(W) into the bias.
- Gathered slot tiles are PE-transposed to [feat, slot] layout, then DVE
  tensor_reduce (max and add) over degree-equal segments accumulates
  agg_max / agg_sum in SBUF [128 feat, 12544 nodes].
- Projection per 128-node chunk: PSUM matmuls Wl_max@agg_max, Wl_mean@agg_sum
  (scaled by 1/deg post-transpose), (Wr_max+Wr_mean)@x, bias, then fused
  log_softmax, DMA out.
"""

import os
import sys

os.environ.setdefault("NEURON_RT_RESET_CORES", "1")
if "/opt/trn_rl_repo" not in sys.path:
    sys.path.insert(0, "/opt/trn_rl_repo")

import numpy as np

import concourse.mybir as mybir
from concourse import bacc, bass, tile
from concourse.masks import make_identity

N_NODES = 100000
D = 128
NCLS = 40
NCORES = 8
NPC = 12500
NPAD = 12544  # 98 * 128
NPROJ = NPAD // 128  # 98
CHUNK = 1536  # gather-chunk slots
IPC = CHUNK // 128  # indirect instrs per chunk
SHIFT = 12.0

last_exec_time_ns = None


def _plan(dst):
    """Per-core degree sort + shared template + chunk/piece layout."""
    core = dst // NPC
    degs = np.zeros((NCORES, NPAD), np.int64)
    orders = np.zeros((NCORES, NPAD), np.int64)
    sdeg = np.zeros((NCORES, NPAD), np.int64)
    for c in range(NCORES):
        dloc = np.bincount(dst[core == c] - c * NPC, minlength=NPC)
        degs[c, :NPC] = dloc
        o = np.argsort(degs[c], kind="stable")
        orders[c] = o
        sdeg[c] = degs[c][o]
    T = sdeg.max(axis=0)

    chunks = []
    p = 0
    while p < NPAD:
        cap = CHUNK
        q = p
        while q < NPAD and T[q] <= cap:
            cap -= T[q]
            q += 1
        chunks.append((p, q))
        p = q

    pieces = []  # per chunk: list of (slot_off, col0, nb, d)
    node_slot_start = np.zeros(NPAD, np.int64)
    for ci, (a, b) in enumerate(chunks):
        node_slot_start[a:b] = ci * CHUNK + np.concatenate(
            [[0], np.cumsum(T[a:b])[:-1]]
        )
        pl = []
        off = 0
        i = a
        while i < b:
            j = i
            while j < b and T[j] == T[i]:
                j += 1
            if T[i] > 0:
                pl.append((int(off), int(i), int(j - i), int(T[i])))
            off += (j - i) * int(T[i])
            i = j
        pieces.append(pl)
    return degs, orders, sdeg, T, chunks, pieces, node_slot_start


def _core_idx(src_c, dstloc_c, order, sdeg_c, node_slot_start, total_slots):
    """int32 slot->xg-row index array for one core (0 = neutral pad row)."""
    pos = np.empty(NPAD, np.int64)
    pos[order] = np.arange(NPAD)
    key = pos[dstloc_c]
    eorder = np.argsort(key, kind="stable")
    s_sorted = src_c[eorder]
    first = np.concatenate([[0], np.cumsum(sdeg_c)[:-1]])
    rank = np.arange(len(s_sorted)) - np.repeat(first, sdeg_c)
    positions = np.repeat(node_slot_start, sdeg_c) + rank
    idx = np.zeros(total_slots, np.int64)
    idx[positions] = s_sorted + 1
    return idx


def _build_program(nchunks, pieces, chunk_ranges):
    nc = bacc.Bacc()
    f32 = mybir.dt.float32
    ncols = nchunks * IPC

    # projection chunk pc is ready once gather chunk ci finalizes all acc
    # cols < (pc+1)*128; emit it right after that chunk's reduces
    proj_after = [[] for _ in range(nchunks)]
    pc = 0
    for ci, (a, b) in enumerate(chunk_ranges):
        while pc < NPROJ and (pc + 1) * 128 <= b:
            proj_after[ci].append(pc)
            pc += 1
    while pc < NPROJ:
        proj_after[-1].append(pc)
        pc += 1

    xg_in = nc.declare_dram_parameter("xg", [N_NODES + 1, D], f32, isOutput=False)
    idx_in = nc.declare_dram_parameter("idx", [128, ncols], mybir.dt.int32,
                                       isOutput=False)
    xT_in = nc.declare_dram_parameter("xT", [D, NPAD], f32, isOutput=False)
    invd_in = nc.declare_dram_parameter("invd", [128, NPROJ], f32, isOutput=False)
    fix_in = nc.declare_dram_parameter("fix", [128, NPROJ, NCLS], f32,
                                       isOutput=False)
    wlmaxT_in = nc.declare_dram_parameter("wlmaxT", [D, NCLS], f32, isOutput=False)
    wlmeanT_in = nc.declare_dram_parameter("wlmeanT", [D, NCLS], f32,
                                           isOutput=False)
    wrcT_in = nc.declare_dram_parameter("wrcT", [D, NCLS], f32, isOutput=False)
    o_out = nc.declare_dram_parameter("out", [NPAD, NCLS], f32, isOutput=True)

    with tile.TileContext(nc) as tc:
        with tc.tile_pool(name="persist", bufs=1) as pers:
            idx_t = pers.tile([128, ncols], mybir.dt.int32)
            invd_t = pers.tile([128, NPROJ], f32)
            fix_t = pers.tile([128, NPROJ, NCLS], f32)
            wlmaxT_t = pers.tile([D, NCLS], f32)
            wlmeanT_t = pers.tile([D, NCLS], f32)
            wrcT_t = pers.tile([D, NCLS], f32)
            ident_t = pers.tile([128, 128], f32)
            acc_max = pers.tile([128, NPAD], f32)
            acc_sum = pers.tile([128, NPAD], f32)

            nc.sync.dma_start(out=idx_t[:, :], in_=idx_in[:, :])
            nc.sync.dma_start(out=invd_t[:, :], in_=invd_in[:, :])
            nc.sync.dma_start(out=fix_t[:, :, :], in_=fix_in[:, :, :])
            nc.sync.dma_start(out=wlmaxT_t[:, :], in_=wlmaxT_in[:, :])
            nc.sync.dma_start(out=wlmeanT_t[:, :], in_=wlmeanT_in[:, :])
            nc.sync.dma_start(out=wrcT_t[:, :], in_=wrcT_in[:, :])
            make_identity(nc, ident_t)
            nc.vector.memset(acc_max[:, :], 0.0)
            nc.vector.memset(acc_sum[:, :], 0.0)

            with tc.tile_pool(name="gath", bufs=4) as gpool, \
                 tc.tile_pool(name="gpsum", bufs=2, space="PSUM") as ppool, \
                 tc.tile_pool(name="proj", bufs=2) as proj, \
                 tc.tile_pool(name="ppsum", bufs=2, space="PSUM") as prps:

                def emit_proj(pc):
                    c0 = pc * 128
                    xT_t = proj.tile([D, 128], f32, name="xTc")
                    nc.sync.dma_start(out=xT_t[:, :], in_=xT_in[:, c0:c0 + 128])

                    # one PSUM bank: [:40, 0:128]=mean mm, [:40,128:256]=
                    # max+root mm, [:,256:296]/[:,296:336]=transposes
                    ps = prps.tile([128, 336], f32, name="ps")
                    nc.tensor.matmul(ps[:NCLS, 0:128], wlmeanT_t[:, :],
                                     acc_sum[:, c0:c0 + 128],
                                     start=True, stop=True)
                    nc.tensor.matmul(ps[:NCLS, 128:256], wlmaxT_t[:, :],
                                     acc_max[:, c0:c0 + 128],
                                     start=True, stop=False)
                    nc.tensor.matmul(ps[:NCLS, 128:256], wrcT_t[:, :],
                                     xT_t[:, :], start=False, stop=True)

                    sA = proj.tile([NCLS, 128], f32, name="sA")
                    sB = proj.tile([NCLS, 128], f32, name="sB")
                    nc.scalar.copy(sA[:, :], ps[:NCLS, 0:128])
                    nc.scalar.copy(sB[:, :], ps[:NCLS, 128:256])
                    nc.tensor.transpose(ps[:, 256:296], sA[:, :],
                                        ident_t[:NCLS, :NCLS])
                    nc.tensor.transpose(ps[:, 296:336], sB[:, :],
                                        ident_t[:NCLS, :NCLS])

                    z = proj.tile([128, NCLS], f32, name="z")
                    nc.vector.tensor_scalar(
                        out=z[:, :], in0=ps[:, 256:296],
                        scalar1=invd_t[:, pc:pc + 1], scalar2=None,
                        op0=mybir.AluOpType.mult,
                    )
                    nc.vector.tensor_tensor(z[:, :], z[:, :], ps[:, 296:336],
                                            mybir.AluOpType.add)
                    nc.vector.tensor_tensor(z[:, :], z[:, :], fix_t[:, pc, :],
                                            mybir.AluOpType.add)

                    m = proj.tile([128, 1], f32, name="m")
                    nc.vector.tensor_reduce(out=m[:, :], in_=z[:, :],
                                            axis=mybir.AxisListType.X,
                                            op=mybir.AluOpType.max)
                    negm = proj.tile([128, 1], f32, name="negm")
                    nc.vector.tensor_scalar(
                        out=negm[:, :], in0=m[:, :], scalar1=-1.0,
                        scalar2=None, op0=mybir.AluOpType.mult,
                    )
                    e = proj.tile([128, NCLS], f32, name="e")
                    se = proj.tile([128, 1], f32, name="se")
                    nc.scalar.activation(
                        e[:, :], z[:, :], mybir.ActivationFunctionType.Exp,
                        bias=negm[:, :1], scale=1.0, accum_out=se[:, :1],
                    )
                    ls = proj.tile([128, 1], f32, name="ls")
                    nc.scalar.activation(ls[:, :], se[:, :],
                                         mybir.ActivationFunctionType.Ln)
                    nc.vector.tensor_tensor(ls[:, :], ls[:, :], m[:, :],
                                            mybir.AluOpType.add)
                    ot = proj.tile([128, NCLS], f32, name="ot")
                    nc.vector.tensor_scalar(
                        out=ot[:, :], in0=z[:, :], scalar1=ls[:, :1],
                        scalar2=None, op0=mybir.AluOpType.subtract,
                    )
                    nc.sync.dma_start(out=o_out[c0:c0 + 128, :], in_=ot[:, :])

                for ci in range(nchunks):
                    g = gpool.tile([128, IPC, D], f32, name="g")
                    for k in range(IPC):
                        col = ci * IPC + k
                        nc.gpsimd.indirect_dma_start(
                            out=g[:, k, :],
                            out_offset=None,
                            in_=xg_in[:, :],
                            in_offset=bass.IndirectOffsetOnAxis(
                                ap=idx_t[:, col:col + 1], axis=0
                            ),
                        )
                    pt = ppool.tile([128, CHUNK], f32, name="pt")
                    for b in range(IPC):
                        nc.tensor.transpose(
                            pt[:, b * 128:(b + 1) * 128], g[:, b, :], ident_t
                        )
                    for (off, col0, nb, dd) in pieces[ci]:
                        seg = pt[:, off:off + nb * dd].rearrange(
                            "p (nb d) -> p nb d", d=dd
                        )
                        nc.vector.tensor_reduce(
                            out=acc_max[:, col0:col0 + nb], in_=seg,
                            axis=mybir.AxisListType.X, op=mybir.AluOpType.max,
                        )
                        nc.vector.tensor_reduce(
                            out=acc_sum[:, col0:col0 + nb], in_=seg,
                            axis=mybir.AxisListType.X, op=mybir.AluOpType.add,
                        )
                    for pc in proj_after[ci]:
                        emit_proj(pc)
    return nc


def kernel(**inputs):
    global last_exec_time_ns
    x = np.asarray(inputs["x"], dtype=np.float32)
    ei = np.asarray(inputs["edge_index"]).astype(np.int64)
    Wl_max = np.asarray(inputs["Wl_max"], dtype=np.float32)
    Wr_max = np.asarray(inputs["Wr_max"], dtype=np.float32)
    b_max = np.asarray(inputs["b_max"], dtype=np.float32)
    Wl_mean = np.asarray(inputs["Wl_mean"], dtype=np.float32)
    Wr_mean = np.asarray(inputs["Wr_mean"], dtype=np.float32)
    b_mean = np.asarray(inputs["b_mean"], dtype=np.float32)

    src, dst = ei[0], ei[1]
    degs, orders, sdeg, T, chunks, pieces, nss = _plan(dst)
    nchunks = len(chunks)
    total_slots = nchunks * CHUNK
    ncols = total_slots // 128

    xg = np.zeros((N_NODES + 1, D), np.float32)
    xg[1:] = x + SHIFT

    rs = SHIFT * (Wl_max.sum(axis=1) + Wl_mean.sum(axis=1))  # [40]
    bias_eff = b_max + b_mean - rs
    wlmaxT = np.ascontiguousarray(Wl_max.T)
    wlmeanT = np.ascontiguousarray(Wl_mean.T)
    wrcT = np.ascontiguousarray((Wr_max + Wr_mean).T)

    core = dst // NPC
    in_maps = []
    for c in range(NCORES):
        msk = core == c
        idx = _core_idx(src[msk], dst[msk] - c * NPC, orders[c], sdeg[c],
                        nss, total_slots)
        idx_t = np.ascontiguousarray(
            idx.reshape(ncols, 128).T).astype(np.int32)

        ids = orders[c]
        real = ids < NPC
        xo = np.zeros((NPAD, D), np.float32)
        xo[real] = x[c * NPC + ids[real]]
        xT = np.ascontiguousarray(xo.T)

        invd = (1.0 / np.maximum(sdeg[c], 1)).astype(np.float32)
        invd_t = np.ascontiguousarray(invd.reshape(NPROJ, 128).T)

        fix = np.tile(bias_eff, (NPAD, 1)).astype(np.float32)
        fix[sdeg[c] == 0] += rs
        fix_t = np.ascontiguousarray(
            fix.reshape(NPROJ, 128, NCLS).transpose(1, 0, 2))

        in_maps.append({
            "xg": xg, "idx": idx_t, "xT": xT, "invd": invd_t, "fix": fix_t,
            "wlmaxT": wlmaxT, "wlmeanT": wlmeanT, "wrcT": wrcT,
        })

    nc = _build_program(nchunks, pieces, chunks)
    nc.compile()

    from concourse.bass_utils import run_bass_kernel_spmd
    res = run_bass_kernel_spmd(nc, in_maps, list(range(NCORES)))
    if os.environ.get("GNN_TRACE", "0") == "1":
        # separate single-core traced run: tracing the 8-core run crashes
        # the exec unit; core 0's time is representative (identical program)
        tr = run_bass_kernel_spmd(nc, in_maps[:1], [0], trace=True)
        last_exec_time_ns = tr.exec_time_ns

    out = np.zeros((N_NODES, NCLS), np.float32)
    for c in range(NCORES):
        o = np.asarray(res.results[c]["out"])
        ids = orders[c]
        real = ids < NPC
        out[c * NPC + ids[real]] = o[real]
    return out



# revision 2
# speedup vs baseline: 1.7278x; 1.7278x over previous
"""GNN message-passing kernel (max+mean aggregation -> linear -> log_softmax)
for Trainium2, 8 NeuronCores, dst-node sharding.

v2: neighbor gather via dma_gather (SWDGE bulk gather, ~16 idx/descriptor)
instead of per-128-row indirect DMAs, removing the Pool-engine descriptor
generation bottleneck (994ns fixed cost per indirect instruction).

dma_gather indices are int16, so the gather table is split into 4 banks of
<=32768 rows. Each core gets its OWN bank tables: a per-core edge coloring
assigns every edge to a bank such that node v receives at most
T_b[p] = (T[p]+3-b)//4 edges from bank b (T = shared degree template).
This keeps the shared SPMD template waste at ~0.4% (vs +63% for a shared
node->bank split).

Stream layout: windows of node positions (each window's per-bank slots
<= CHUNK); per window, bank0's segmented reduce writes acc directly,
banks 1-3 reduce into tmp tiles and combine (max/add) into acc.
Projection per 128-node chunk as windows complete: PSUM matmuls
Wl_mean@agg_sum, Wl_max@agg_max + (Wr_max+Wr_mean)@x, bias, fused
log_softmax, DMA out.
"""

import os
import sys

os.environ.setdefault("NEURON_RT_RESET_CORES", "1")
if "/opt/trn_rl_repo" not in sys.path:
    sys.path.insert(0, "/opt/trn_rl_repo")

import numpy as np

import concourse.mybir as mybir
from concourse import bacc, bass, tile
from concourse.masks import make_identity

N_NODES = 100000
D = 128
NCLS = 40
NCORES = 8
NPC = 12500
NPAD = 12544  # 98 * 128
NPROJ = NPAD // 128  # 98
NBANK = 4
TBL_ROWS = 32768
CHUNK = 1024  # max slots per (window, bank) gather (HW dma_gather limit)
SHIFT = 12.0

last_exec_time_ns = None


def _plan_template(dst):
    """Shared template: per-core degree sort, T = max sorted degree,
    bank splits, windows, per-chunk pieces, idx column layout."""
    core = dst // NPC
    degs = np.zeros((NCORES, NPAD), np.int64)
    orders = np.zeros((NCORES, NPAD), np.int64)
    for c in range(NCORES):
        degs[c, :NPC] = np.bincount(dst[core == c] - c * NPC, minlength=NPC)
        orders[c] = np.argsort(degs[c], kind="stable")
    sdeg = np.take_along_axis(degs, orders, axis=1)
    T = sdeg.max(axis=0)
    Tb = np.stack([(T + 3 - b) // 4 for b in range(NBANK)])  # [4, NPAD]

    # windows cut by bank0 (largest) slot count
    windows = []
    pa = 0
    cum = 0
    for p in range(NPAD):
        if cum + Tb[0, p] > CHUNK:
            windows.append((pa, p))
            pa = p
            cum = 0
        cum += Tb[0, p]
    windows.append((pa, NPAD))

    # chunks: (window, bank) with active suffix, pieces, idx col offset
    chunks = []  # dicts
    col0 = 0
    for (pa, pb) in windows:
        for b in range(NBANK):
            tb = Tb[b, pa:pb]
            L = int(tb.sum())
            if L == 0:
                chunks.append(dict(b=b, pa=pa, pb=pb, qa=pb, L=0, Lpad=0,
                                   col0=col0, pieces=[]))
                continue
            nz = np.flatnonzero(tb > 0)
            qa = pa + int(nz[0])
            Lpad = -(-L // 128) * 128
            # pieces: runs of equal d over [qa, pb)
            dvals = Tb[b, qa:pb]
            pieces = []
            off = 0
            i = 0
            while i < len(dvals):
                j = i
                while j < len(dvals) and dvals[j] == dvals[i]:
                    j += 1
                d = int(dvals[i])
                pieces.append((int(off), int(qa + i), int(j - i), d))
                off += (j - i) * d
                i = j
            chunks.append(dict(b=b, pa=pa, pb=pb, qa=qa, L=L, Lpad=Lpad,
                               col0=col0, pieces=pieces))
            col0 += Lpad // 16
    ncols = col0

    # projection readiness: after window w's chunks, positions < pb combined
    proj_after = []  # per window: list of proj chunk ids
    pc = 0
    for wi, (pa, pb) in enumerate(windows):
        lst = []
        while pc < NPROJ and (pc + 1) * 128 <= pb:
            lst.append(pc)
            pc += 1
        proj_after.append(lst)
    while pc < NPROJ:
        proj_after[-1].append(pc)
        pc += 1
    return degs, orders, sdeg, T, Tb, windows, chunks, ncols, proj_after


def _color_core(src_c, dstloc, pos, Tb):
    """Assign each edge a bank: d_b(v) <= Tb[b, pos[v]], minimizing distinct
    (src, bank) pairs. Returns per-edge bank."""
    ne = len(src_c)
    p_e = pos[dstloc]
    rem = Tb.copy()
    bank = np.full(ne, -1, np.int8)

    order = np.argsort(src_c, kind="stable")
    s_sorted = src_c[order]
    uniq, start, cnt = np.unique(s_sorted, return_index=True,
                                 return_counts=True)
    src_rank = np.argsort(-cnt, kind="stable")
    rows_used = np.zeros(NBANK, np.int64)
    multi = src_rank[cnt[src_rank] >= 2]
    singles = src_rank[cnt[src_rank] == 1]

    for si in multi:
        a, k = start[si], cnt[si]
        eidx = order[a:a + k]
        ps = p_e[eidx]
        up, ucnt = np.unique(ps, return_counts=True)
        fits = (rem[:, up] >= ucnt).all(axis=1)
        if fits.any():
            fb = np.flatnonzero(fits)
            b = fb[np.argmin(rows_used[fb])]
            bank[eidx] = b
            np.subtract.at(rem[b], up, ucnt)
            rows_used[b] += 1
        else:
            used = set()
            for e in eidx:
                pe = p_e[e]
                b = int(np.argmax(rem[:, pe]))
                bank[e] = b
                rem[b, pe] -= 1
                used.add(b)
            rows_used[list(used)] += 1

    se = order[start[singles]]
    for chunk in np.array_split(se, max(1, len(se) // 20000)):
        pe = p_e[chunk]
        b = np.argmax(rem[:, pe], axis=0)
        np.subtract.at(rem, (b, pe), 1)
        bank[chunk] = b
        bad = np.argwhere(rem < 0)
        for bb, pp in bad:
            over = -rem[bb, pp]
            cand = chunk[(pe == pp) & (bank[chunk] == bb)]
            for e in cand[:over]:
                nb = int(np.argmax(rem[:, pp]))
                bank[e] = nb
                rem[nb, pp] -= 1
                rem[bb, pp] += 1
    assert (bank >= 0).all()
    assert (rem >= 0).all()
    return bank


def _core_data(x, src_c, dstloc, pos, Tb, chunks, ncols):
    """Bank tables (fp32, shifted, row0=0) and wrapped int16 idx array."""
    bank = _color_core(src_c, dstloc, pos, Tb)
    p_e = pos[dstloc]

    tables = []
    rows = []
    for b in range(NBANK):
        used = np.unique(src_c[bank == b])
        assert len(used) + 1 <= TBL_ROWS, f"bank {b} overflow: {len(used)}"
        tbl = np.zeros((TBL_ROWS, D), np.float32)
        tbl[1:1 + len(used)] = x[used] + SHIFT
        tables.append(tbl)
        rows.append(used)

    idx_flat = np.zeros(ncols * 16, np.int16)
    for ch in chunks:
        b, qa, pb, L, Lpad, col0 = (ch["b"], ch["qa"], ch["pb"], ch["L"],
                                    ch["Lpad"], ch["col0"])
        if L == 0:
            continue
        m = (bank == b) & (p_e >= qa) & (p_e < pb)
        if m.any():
            eidx = np.flatnonzero(m)
            pe = p_e[eidx]
            o = np.argsort(pe, kind="stable")
            eidx = eidx[o]
            pe = pe[o]
            # rank within pos group
            grp_start = np.concatenate([[0], np.flatnonzero(np.diff(pe)) + 1])
            sizes = np.diff(np.concatenate([grp_start, [len(pe)]]))
            rank = np.arange(len(pe)) - np.repeat(grp_start, sizes)
            sec = np.zeros(pb - qa, np.int64)
            sec[1:] = np.cumsum(Tb[b, qa:pb - 1])
            slots = sec[pe - qa] + rank
            row_id = np.searchsorted(rows[b], src_c[eidx]) + 1
            blk = np.zeros(Lpad, np.int16)
            blk[slots] = row_id.astype(np.int16)
        else:
            blk = np.zeros(Lpad, np.int16)
        idx_flat[col0 * 16: col0 * 16 + Lpad] = blk
    # wrap: idx i of each chunk at [i%16, col0 + i//16]
    wrapped = idx_flat.reshape(ncols, 16).T  # [16, ncols]
    idx_arr = np.tile(wrapped, (8, 1)).astype(np.int16)  # [128, ncols]
    return tables, idx_arr


def _build_program(chunks_by_window, proj_after, ncols):
    nc = bacc.Bacc(num_swdge_queues=4)
    f32 = mybir.dt.float32
    i16 = mybir.dt.int16

    tbl_in = [nc.declare_dram_parameter(f"tbl{b}", [TBL_ROWS, D], f32,
                                        isOutput=False) for b in range(NBANK)]
    idx_in = nc.declare_dram_parameter("idx", [128, ncols], i16,
                                       isOutput=False)
    xT_in = nc.declare_dram_parameter("xT", [D, NPAD], f32, isOutput=False)
    invd_in = nc.declare_dram_parameter("invd", [128, NPROJ], f32,
                                        isOutput=False)
    fix_in = nc.declare_dram_parameter("fix", [128, NPROJ, NCLS], f32,
                                       isOutput=False)
    wlmaxT_in = nc.declare_dram_parameter("wlmaxT", [D, NCLS], f32,
                                          isOutput=False)
    wlmeanT_in = nc.declare_dram_parameter("wlmeanT", [D, NCLS], f32,
                                           isOutput=False)
    wrcT_in = nc.declare_dram_parameter("wrcT", [D, NCLS], f32,
                                        isOutput=False)
    o_out = nc.declare_dram_parameter("out", [NPAD, NCLS], f32, isOutput=True)

    with tile.TileContext(nc) as tc:
        with tc.tile_pool(name="persist", bufs=1) as pers:
            idx_t = pers.tile([128, ncols], i16)
            invd_t = pers.tile([128, NPROJ], f32)
            fix_t = pers.tile([128, NPROJ, NCLS], f32)
            wlmaxT_t = pers.tile([D, NCLS], f32)
            wlmeanT_t = pers.tile([D, NCLS], f32)
            wrcT_t = pers.tile([D, NCLS], f32)
            ident_t = pers.tile([128, 128], f32)
            acc_max = pers.tile([128, NPAD], f32)
            acc_sum = pers.tile([128, NPAD], f32)

            nc.sync.dma_start(out=idx_t[:, :], in_=idx_in[:, :])
            nc.sync.dma_start(out=invd_t[:, :], in_=invd_in[:, :])
            nc.sync.dma_start(out=fix_t[:, :, :], in_=fix_in[:, :, :])
            nc.sync.dma_start(out=wlmaxT_t[:, :], in_=wlmaxT_in[:, :])
            nc.sync.dma_start(out=wlmeanT_t[:, :], in_=wlmeanT_in[:, :])
            nc.sync.dma_start(out=wrcT_t[:, :], in_=wrcT_in[:, :])
            make_identity(nc, ident_t)
            nc.vector.memset(acc_max[:, :], 0.0)
            nc.vector.memset(acc_sum[:, :], 0.0)

            gather_seq = [0]

            with tc.tile_pool(name="gath", bufs=6) as gpool, \
                 tc.tile_pool(name="gpsum", bufs=3, space="PSUM") as ppool, \
                 tc.tile_pool(name="tmp", bufs=4) as tpool, \
                 tc.tile_pool(name="proj", bufs=2) as proj, \
                 tc.tile_pool(name="ppsum", bufs=2, space="PSUM") as prps:

                def emit_proj(pc):
                    c0 = pc * 128
                    xT_t = proj.tile([D, 128], f32, name="xTc")
                    nc.sync.dma_start(out=xT_t[:, :], in_=xT_in[:, c0:c0 + 128])

                    ps = prps.tile([128, 336], f32, name="ps")
                    nc.tensor.matmul(ps[:NCLS, 0:128], wlmeanT_t[:, :],
                                     acc_sum[:, c0:c0 + 128],
                                     start=True, stop=True)
                    nc.tensor.matmul(ps[:NCLS, 128:256], wlmaxT_t[:, :],
                                     acc_max[:, c0:c0 + 128],
                                     start=True, stop=False)
                    nc.tensor.matmul(ps[:NCLS, 128:256], wrcT_t[:, :],
                                     xT_t[:, :], start=False, stop=True)

                    sA = proj.tile([NCLS, 128], f32, name="sA")
                    sB = proj.tile([NCLS, 128], f32, name="sB")
                    nc.scalar.copy(sA[:, :], ps[:NCLS, 0:128])
                    nc.scalar.copy(sB[:, :], ps[:NCLS, 128:256])
                    nc.tensor.transpose(ps[:, 256:296], sA[:, :],
                                        ident_t[:NCLS, :NCLS])
                    nc.tensor.transpose(ps[:, 296:336], sB[:, :],
                                        ident_t[:NCLS, :NCLS])

                    z = proj.tile([128, NCLS], f32, name="z")
                    nc.vector.tensor_scalar(
                        out=z[:, :], in0=ps[:, 256:296],
                        scalar1=invd_t[:, pc:pc + 1], scalar2=None,
                        op0=mybir.AluOpType.mult,
                    )
                    nc.vector.tensor_tensor(z[:, :], z[:, :], ps[:, 296:336],
                                            mybir.AluOpType.add)
                    nc.vector.tensor_tensor(z[:, :], z[:, :], fix_t[:, pc, :],
                                            mybir.AluOpType.add)

                    m = proj.tile([128, 1], f32, name="m")
                    nc.vector.tensor_reduce(out=m[:, :], in_=z[:, :],
                                            axis=mybir.AxisListType.X,
                                            op=mybir.AluOpType.max)
                    negm = proj.tile([128, 1], f32, name="negm")
                    nc.vector.tensor_scalar(
                        out=negm[:, :], in0=m[:, :], scalar1=-1.0,
                        scalar2=None, op0=mybir.AluOpType.mult,
                    )
                    e = proj.tile([128, NCLS], f32, name="e")
                    se = proj.tile([128, 1], f32, name="se")
                    nc.scalar.activation(
                        e[:, :], z[:, :], mybir.ActivationFunctionType.Exp,
                        bias=negm[:, :1], scale=1.0, accum_out=se[:, :1],
                    )
                    ls = proj.tile([128, 1], f32, name="ls")
                    nc.scalar.activation(ls[:, :], se[:, :],
                                         mybir.ActivationFunctionType.Ln)
                    nc.vector.tensor_tensor(ls[:, :], ls[:, :], m[:, :],
                                            mybir.AluOpType.add)
                    ot = proj.tile([128, NCLS], f32, name="ot")
                    nc.vector.tensor_scalar(
                        out=ot[:, :], in0=z[:, :], scalar1=ls[:, :1],
                        scalar2=None, op0=mybir.AluOpType.subtract,
                    )
                    nc.sync.dma_start(out=o_out[c0:c0 + 128, :], in_=ot[:, :])

                for wi, wchunks in enumerate(chunks_by_window):
                    for ch in wchunks:
                        if ch["L"] == 0:
                            continue
                        b, qa, pb = ch["b"], ch["qa"], ch["pb"]
                        Lpad, col0 = ch["Lpad"], ch["col0"]
                        G = Lpad // 128
                        Q = pb - qa
                        g = gpool.tile([128, G, D], f32, name="g")
                        nc.gpsimd.dma_gather(
                            g[:, :, :], tbl_in[b][:, :],
                            idx_t[:, col0:col0 + Lpad // 16],
                            Lpad, Lpad, D,
                            queue_num=gather_seq[0] % 4,
                        )
                        gather_seq[0] += 1
                        pt = ppool.tile([128, Lpad], f32, name="pt")
                        for k in range(G):
                            nc.tensor.transpose(
                                pt[:, k * 128:(k + 1) * 128], g[:, k, :],
                                ident_t,
                            )
                        if b == 0:
                            for (off, pos0, nbn, dd) in ch["pieces"]:
                                seg = pt[:, off:off + nbn * dd].rearrange(
                                    "p (nb d) -> p nb d", d=dd)
                                nc.vector.tensor_reduce(
                                    out=acc_max[:, pos0:pos0 + nbn], in_=seg,
                                    axis=mybir.AxisListType.X,
                                    op=mybir.AluOpType.max)
                                nc.vector.tensor_reduce(
                                    out=acc_sum[:, pos0:pos0 + nbn], in_=seg,
                                    axis=mybir.AxisListType.X,
                                    op=mybir.AluOpType.add)
                        else:
                            tm = tpool.tile([128, Q], f32, name="tm")
                            ts = tpool.tile([128, Q], f32, name="ts")
                            for (off, pos0, nbn, dd) in ch["pieces"]:
                                q0 = pos0 - qa
                                seg = pt[:, off:off + nbn * dd].rearrange(
                                    "p (nb d) -> p nb d", d=dd)
                                nc.vector.tensor_reduce(
                                    out=tm[:, q0:q0 + nbn], in_=seg,
                                    axis=mybir.AxisListType.X,
                                    op=mybir.AluOpType.max)
                                nc.vector.tensor_reduce(
                                    out=ts[:, q0:q0 + nbn], in_=seg,
                                    axis=mybir.AxisListType.X,
                                    op=mybir.AluOpType.add)
                            nc.vector.tensor_tensor(
                                acc_max[:, qa:pb], acc_max[:, qa:pb],
                                tm[:, :], mybir.AluOpType.max)
                            nc.vector.tensor_tensor(
                                acc_sum[:, qa:pb], acc_sum[:, qa:pb],
                                ts[:, :], mybir.AluOpType.add)
                    for pc in proj_after[wi]:
                        emit_proj(pc)
    return nc


def kernel(**inputs):
    global last_exec_time_ns
    x = np.asarray(inputs["x"], dtype=np.float32)
    ei = np.asarray(inputs["edge_index"]).astype(np.int64)
    Wl_max = np.asarray(inputs["Wl_max"], dtype=np.float32)
    Wr_max = np.asarray(inputs["Wr_max"], dtype=np.float32)
    b_max = np.asarray(inputs["b_max"], dtype=np.float32)
    Wl_mean = np.asarray(inputs["Wl_mean"], dtype=np.float32)
    Wr_mean = np.asarray(inputs["Wr_mean"], dtype=np.float32)
    b_mean = np.asarray(inputs["b_mean"], dtype=np.float32)

    src, dst = ei[0], ei[1]
    (degs, orders, sdeg, T, Tb, windows, chunks, ncols,
     proj_after) = _plan_template(dst)

    # group chunks by window for program emission
    chunks_by_window = []
    ci = 0
    for _ in windows:
        chunks_by_window.append(chunks[ci:ci + NBANK])
        ci += NBANK

    rs = SHIFT * (Wl_max.sum(axis=1) + Wl_mean.sum(axis=1))  # [40]
    bias_eff = b_max + b_mean - rs
    wlmaxT = np.ascontiguousarray(Wl_max.T)
    wlmeanT = np.ascontiguousarray(Wl_mean.T)
    wrcT = np.ascontiguousarray((Wr_max + Wr_mean).T)

    core = dst // NPC
    in_maps = []
    for c in range(NCORES):
        msk = core == c
        src_c = src[msk]
        dstloc = dst[msk] - c * NPC
        pos = np.empty(NPAD, np.int64)
        pos[orders[c]] = np.arange(NPAD)
        tables, idx_arr = _core_data(x, src_c, dstloc, pos, Tb, chunks, ncols)

        ids = orders[c]
        real = ids < NPC
        xo = np.zeros((NPAD, D), np.float32)
        xo[real] = x[c * NPC + ids[real]]
        xT = np.ascontiguousarray(xo.T)

        invd = (1.0 / np.maximum(sdeg[c], 1)).astype(np.float32)
        invd_t = np.ascontiguousarray(invd.reshape(NPROJ, 128).T)

        fix = np.tile(bias_eff, (NPAD, 1)).astype(np.float32)
        fix[sdeg[c] == 0] += rs
        fix_t = np.ascontiguousarray(
            fix.reshape(NPROJ, 128, NCLS).transpose(1, 0, 2))

        im = {"idx": idx_arr, "xT": xT, "invd": invd_t, "fix": fix_t,
              "wlmaxT": wlmaxT, "wlmeanT": wlmeanT, "wrcT": wrcT}
        for b in range(NBANK):
            im[f"tbl{b}"] = tables[b]
        in_maps.append(im)

    nc = _build_program(chunks_by_window, proj_after, ncols)
    nc.compile()

    from concourse.bass_utils import run_bass_kernel_spmd
    res = run_bass_kernel_spmd(nc, in_maps, list(range(NCORES)))
    if os.environ.get("GNN_TRACE", "0") == "1":
        tr = run_bass_kernel_spmd(nc, in_maps[:1], [0], trace=True)
        last_exec_time_ns = tr.exec_time_ns

    out = np.zeros((N_NODES, NCLS), np.float32)
    for c in range(NCORES):
        o = np.asarray(res.results[c]["out"])
        ids = orders[c]
        real = ids < NPC
        out[c * NPC + ids[real]] = o[real]
    return out
